# revision 6
# baseline (speedup 1.0000x reference)
"""Trainium2 Bass kernel for nn_CHARM_40200893891073.

Reference math: the Conv1d branch is dead code — the output is
    remap = exp(rowsum(emb) [:,None] * colsum(emb) [None,:]) / D
broadcast over the batch dim:  out[b, c, d] = remap[c, d]  for all b.

Strategy (data-parallel over batch, 8 cores):
  Each core computes remap [64, 256] on-chip from the replicated
  emb_weight and writes its [64, 64, 256] batch shard (4 MiB) to DRAM
  with a single broadcast-source DMA.  Per-core HBM traffic is just the
  output write, which is the memory roofline for this problem.

On-chip pipeline (raw bass; Tile's tail drain doesn't compile on this
walrus build):
  1. DMA emb [64,256] into SBUF twice (partitions 0-63 and 64-127).
  2. DVE: rowsum over the free axis -> rs [128, 1].
  3. PE:  ones[64,128]^T @ emb -> psum [128, 256] = colsum replicated
     into every partition (engines cannot partition-broadcast reads,
     so the matmul does the replication).
  4. ACT: remap[p, d] = Exp(psum[p, d] * rs[p] - ln(D))  (per-partition
     scale AP, input straight from PSUM).
  5. One DMA: remap [128, 256] with a zero-stride repeat dim -> the
     full [64, 64, 256] shard (partition p = (b%2)*64 + c).
"""

import numpy as np

B, CH, L, D = 512, 64, 1024, 256
NCORES = 8
BS = B // NCORES  # batches per core

_CACHE: dict = {}


def _build_nc():
    import concourse.bass as bass
    import concourse.mybir as mybir

    nc = bass.Bass()
    emb = nc.dram_tensor("emb_weight", [CH, D], mybir.dt.float32, kind="ExternalInput")
    out = nc.dram_tensor("out", [BS, CH, D], mybir.dt.float32, kind="ExternalOutput")

    ln_d = float(np.log(float(D)))

    with (
        nc.sbuf_tensor([128, D], mybir.dt.float32) as emb_sb,
        nc.sbuf_tensor([128, 1], mybir.dt.float32) as rs_sb,
        nc.sbuf_tensor([64, 128], mybir.dt.float32) as ones_sb,
        nc.sbuf_tensor([128, 1], mybir.dt.float32) as bias_sb,
        nc.sbuf_tensor([128, D], mybir.dt.float32) as remap_sb,
        nc.psum_tensor([128, D], mybir.dt.float32) as psum_cs,
        nc.semaphore("dma_in") as dma_in,
        nc.semaphore("s_ones") as s_ones,
        nc.semaphore("s_red") as s_red,
        nc.semaphore("s_cs") as s_cs,
        nc.semaphore("s_act") as s_act,
        nc.semaphore("dma_out") as dma_out,
        nc.Block() as block,
    ):

        @block.sync
        def _(sync):
            sync.dma_start(out=emb_sb[0:CH, :], in_=emb[:, :]).then_inc(dma_in, 16)
            sync.dma_start(out=emb_sb[CH : 2 * CH, :], in_=emb[:, :]).then_inc(
                dma_in, 16
            )
            sync.wait_ge(s_act, 1)
            # out[b, c, d] with b = b2*2 + bl  <-  remap_sb[bl*64 + c, d]
            # dest dim0 (bl c) merges to stride 256, count 128.
            sync.dma_start(
                out=out.rearrange("(b2 bl) c d -> (bl c) b2 d", bl=2),
                in_=remap_sb[:, :].unsqueeze(1).to_broadcast((128, BS // 2, D)),
            ).then_inc(dma_out, 16)
            sync.wait_ge(dma_out, 16)

        @block.vector
        def _(vector):
            vector.memset(ones_sb[:, :], 1.0).then_inc(s_ones, 1)
            vector.memset(bias_sb[:, :], -ln_d)
            vector.wait_ge(dma_in, 32)
            vector.reduce_sum(
                out=rs_sb[:, 0:1], in_=emb_sb[:, :], axis=mybir.AxisListType.X
            ).then_inc(s_red, 1)

        @block.tensor
        def _(tensor):
            tensor.wait_ge(s_ones, 1)
            tensor.wait_ge(dma_in, 16)
            # out[p, d] = sum_c emb[c, d] = colsum[d], for every partition p
            tensor.matmul(
                psum_cs[:, :],
                lhsT=ones_sb[:, :],
                rhs=emb_sb[0:CH, :],
                start=True,
                stop=True,
            ).then_inc(s_cs, 1)

        @block.scalar
        def _(scalar):
            scalar.wait_ge(s_red, 1)
            scalar.wait_ge(s_cs, 1)
            scalar.activation(
                out=remap_sb[:, :],
                in_=psum_cs[:, :],
                func=mybir.ActivationFunctionType.Exp,
                bias=bias_sb[:, 0:1],
                scale=rs_sb[:, 0:1],
            ).then_inc(s_act, 1)

    return nc


LAST_RESULTS = None


def kernel(**inputs) -> np.ndarray:
    global LAST_RESULTS
    from concourse.bass_utils import run_bass_kernel_spmd

    emb = np.ascontiguousarray(inputs["emb_weight"], dtype=np.float32)
    assert emb.shape == (CH, D)

    if "nc" not in _CACHE:
        _CACHE["nc"] = _build_nc()
    nc = _CACHE["nc"]

    in_maps = [{"emb_weight": emb} for _ in range(NCORES)]
    res = run_bass_kernel_spmd(nc, in_maps, core_ids=list(range(NCORES)))
    LAST_RESULTS = res
    out = np.concatenate([r["out"] for r in res.results], axis=0)
    assert out.shape == (B, CH, D)
    return np.ascontiguousarray(out, dtype=np.float32)
